# revision 7
# baseline (speedup 1.0000x reference)
"""Trainium2 Bass kernel for nn_F2FBlock (2-layer SAGEConv GNN block).

Full inputs in, full output out. Internally: nodes sharded 6250/core across
8 NeuronCores (padded to 6272 = 49*128), edges sharded by dst ownership and
sorted by dst into 49 dst-tiles x CPT chunks of 128 edges.

Aggregation: per dst tile ONE InstDMAGatherAnt (gpsimd dma_gather) fetches
cpt*128 pre-transformed rows from the pair-packed bf16 table
([NPAD/2, 256] = two nodes per 512B row; int16 row ids fit), then a single
fused is_equal builds 2*cpt parity-masked one-hot matrices and 2*cpt
PE matmuls segment-sum the correct-parity halves into PSUM. The two conv
layers exchange node features with an on-device AllGather.

reference math:
    shortcut = x @ sc_w.T + sc_b
    h = gelu(x @ dp_w.T + dp_b)
    h = mean_agg(h)@g1_lw.T + g1_lb + h@g1_rw.T          (SAGEConv 1)
    h = gelu(LN(h, n1_g, n1_b))
    h = mean_agg(h)@g2_lw.T + g2_lb + h@g2_rw.T          (SAGEConv 2)
    h = LN(h, n2_g, n2_b)
    out = gelu(h + shortcut)
where mean_agg(h)[i] = mean over {h[src] : (src,dst=i) in edges}.
Linearity lets us aggregate hl = h @ w_l.T and scale by 1/deg after.
"""

import numpy as np
import ml_dtypes

import concourse.bass as bass
import concourse.bacc as bacc
import concourse.tile as tile
import concourse.mybir as mybir
from concourse.masks import make_identity

P = 128
D = 128
N = 50000
NCORE = 8
OWN = N // NCORE            # 6250 owned nodes per core
NT = (OWN + P - 1) // P     # 49 dst tiles per core
SLAB = NT * P               # 6272 padded rows per core
NPAD = SLAB * NCORE         # 50176 rows in gathered tables
EPS = 1e-5

F32 = mybir.dt.float32
BF16 = mybir.dt.bfloat16
I16 = mybir.dt.int16
AF = mybir.ActivationFunctionType
ALU = mybir.AluOpType


def _build_nc(cpt: int, reps: int = 1):
    """Build the Bass module. cpt = chunks (of 128 edges) per dst tile.
    reps > 1 repeats the whole computation (for timing differentials)."""
    nidx = cpt * P              # gather slots per dst tile
    nc = bacc.Bacc("TRN2", target_bir_lowering=False, debug=False,
                   num_devices=NCORE, num_swdge_queues=1)

    # ---- I/O ----
    x_t = nc.dram_tensor("x_t", [P, SLAB], F32, kind="ExternalInput")
    src_idx = nc.dram_tensor("src_idx", [P, NT * cpt * 8], I16,
                             kind="ExternalInput")
    dl2_in = nc.dram_tensor("dl2_in", [P, NT * 2 * cpt], BF16,
                            kind="ExternalInput")
    iota2_in = nc.dram_tensor("iota2_in", [P, 2 * cpt * P], BF16,
                              kind="ExternalInput")
    inv_cnt = nc.dram_tensor("inv_cnt", [P, NT], F32, kind="ExternalInput")
    # weight matrices, already transposed to [fin, fout] on host
    w_names = ["w_dp", "w_sc", "w_g1l", "w_g1r", "w_g2l", "w_g2r"]
    w_in = {n: nc.dram_tensor(n, [D, D], F32, kind="ExternalInput") for n in w_names}
    dp_b = nc.dram_tensor("dp_b", [D, 1], F32, kind="ExternalInput")
    # feature-axis vectors replicated to [P, D] on host
    r_names = ["sc_b", "g1_lb", "g2_lb", "n1_g", "n1_b", "n2_g", "n2_b"]
    r_in = {n: nc.dram_tensor(n, [P, D], F32, kind="ExternalInput") for n in r_names}
    out = nc.dram_tensor("out", [SLAB, D], F32, kind="ExternalOutput")

    with tile.TileContext(nc) as tc:
        with (
            tc.tile_pool(name="const", bufs=1) as cp,
            tc.tile_pool(name="work", bufs=4) as wp,
            tc.tile_pool(name="msgs", bufs=4) as mp,
            tc.tile_pool(name="oneh", bufs=4) as op_,
            tc.tile_pool(name="small", bufs=4) as sp,
            tc.tile_pool(name="psA", bufs=2, space="PSUM") as pA,
            tc.tile_pool(name="psB", bufs=4, space="PSUM") as pB,
            tc.tile_pool(name="dram", bufs=1, space="DRAM") as dp_,
        ):
            # ---- constants into SBUF ----
            xt_s = cp.tile([P, SLAB], F32, tag="xt")
            nc.sync.dma_start(out=xt_s[:], in_=x_t[:])
            si_s = cp.tile([P, NT * cpt * 8], I16, tag="si")
            nc.sync.dma_start(out=si_s[:], in_=src_idx[:])
            dl2_s = cp.tile([P, NT * 2 * cpt], BF16, tag="dl2")
            nc.sync.dma_start(out=dl2_s[:], in_=dl2_in[:])
            io2_s = cp.tile([P, 2 * cpt * P], BF16, tag="io2")
            nc.sync.dma_start(out=io2_s[:], in_=iota2_in[:])
            ic_s = cp.tile([P, NT], F32, tag="ic")
            nc.sync.dma_start(out=ic_s[:], in_=inv_cnt[:])
            w_s = {}
            for n in w_names:
                w_s[n] = cp.tile([D, D], F32, tag=n, name=n)
                nc.sync.dma_start(out=w_s[n][:], in_=w_in[n][:])
            dpb_s = cp.tile([D, 1], F32, tag="dpb")
            nc.sync.dma_start(out=dpb_s[:], in_=dp_b[:])
            r_s = {}
            for n in r_names:
                r_s[n] = cp.tile([P, D], F32, tag=n, name=n)
                nc.sync.dma_start(out=r_s[n][:], in_=r_in[n][:])
            ident = cp.tile([P, P], F32, tag="ident")
            make_identity(nc, ident[:])

            # internal DRAM state (tables pair-packed: [NPAD//2, 256])
            hl1slab = dp_.tile([SLAB, D], BF16)
            hl1full = dp_.tile([NPAD // 2, 2 * D], BF16)
            hl2slab = dp_.tile([SLAB, D], BF16)
            hl2full = dp_.tile([NPAD // 2, 2 * D], BF16)
            h0r_d = dp_.tile([SLAB, D], F32)
            shct_d = dp_.tile([SLAB, D], F32)

            def layer_norm(h, gamma_t, beta_t):
                """LN over free dim of node-major h [P, D]."""
                scratch = wp.tile([P, D], F32, tag="lnscr")
                sumsq = sp.tile([P, 1], F32, tag="sumsq")
                nc.scalar.activation(out=scratch[:], in_=h[:], func=AF.Square,
                                     accum_out=sumsq[:])
                ssum = sp.tile([P, 1], F32, tag="ssum")
                nc.vector.tensor_reduce(out=ssum[:], in_=h[:],
                                        axis=mybir.AxisListType.X, op=ALU.add)
                mu = sp.tile([P, 1], F32, tag="mu")
                nc.vector.tensor_scalar_mul(out=mu[:], in0=ssum[:], scalar1=1.0 / D)
                musq = sp.tile([P, 1], F32, tag="musq")
                nc.vector.tensor_tensor(out=musq[:], in0=mu[:], in1=mu[:], op=ALU.mult)
                var = sp.tile([P, 1], F32, tag="var")
                nc.vector.scalar_tensor_tensor(out=var[:], in0=sumsq[:],
                                               scalar=1.0 / D, in1=musq[:],
                                               op0=ALU.mult, op1=ALU.subtract)
                nc.vector.tensor_scalar_add(out=var[:], in0=var[:], scalar1=EPS)
                sd = sp.tile([P, 1], F32, tag="sd")
                nc.scalar.activation(out=sd[:], in_=var[:], func=AF.Sqrt)
                rstd = sp.tile([P, 1], F32, tag="rstd")
                nc.vector.reciprocal(out=rstd[:], in_=sd[:])
                nmr = sp.tile([P, 1], F32, tag="nmr")
                nc.vector.scalar_tensor_tensor(out=nmr[:], in0=mu[:], scalar=-1.0,
                                               in1=rstd[:], op0=ALU.mult, op1=ALU.mult)
                hn = wp.tile([P, D], F32, tag="hn")
                nc.scalar.activation(out=hn[:], in_=h[:], func=AF.Identity,
                                     scale=rstd[:], bias=nmr[:])
                hg = wp.tile([P, D], F32, tag="hg")
                nc.vector.tensor_tensor(out=hg[:], in0=hn[:], in1=gamma_t[:], op=ALU.mult)
                nc.vector.tensor_tensor(out=hg[:], in0=hg[:], in1=beta_t[:], op=ALU.add)
                return hg

            def aggregate(table, t):
                """Mean-aggregate dst tile t from the pair-packed bf16 table;
                returns mean-scaled f32 sbuf tile [P, D]."""
                msgs = mp.tile([P, cpt * 2 * D], BF16, tag="msgs")
                nc.gpsimd.dma_gather(
                    out_ap=msgs[:].rearrange("p (c e) -> p c e", e=2 * D),
                    in_ap=table[:],
                    idxs_ap=si_s[:, t * cpt * 8:(t + 1) * cpt * 8],
                    num_idxs=nidx,
                    num_idxs_reg=nidx,
                    elem_size=2 * D,
                    queue_num=0,
                    single_packet=False,
                )
                w_all = op_.tile([P, 2 * cpt * P], BF16, tag="oneh")
                nc.vector.tensor_tensor(
                    out=w_all[:].rearrange("p (c e) -> p c e", e=P),
                    in0=dl2_s[:, t * 2 * cpt:(t + 1) * 2 * cpt]
                        .to_broadcast([P, 2 * cpt, P]),
                    in1=io2_s[:].rearrange("p (c e) -> p c e", e=P),
                    op=ALU.is_equal)
                ps = pA.tile([P, D], F32, space="PSUM", tag="agg")
                for m in range(2 * cpt):
                    nc.tensor.matmul(ps[:], lhsT=w_all[:, m * P:(m + 1) * P],
                                     rhs=msgs[:, m * P:(m + 1) * P],
                                     start=(m == 0), stop=(m == 2 * cpt - 1))
                h = wp.tile([P, D], F32, tag="hagg")
                nc.scalar.mul(out=h[:], in_=ps[:], mul=ic_s[:, t:t + 1])
                return h

            for _rep in range(reps):
                # ---- phase B1: hl1 chain only (feeds AllGather 1 asap) ----
                h0fm_tiles = []
                for i in range(NT):
                    xt_i = xt_s[:, i * P:(i + 1) * P]
                    ph = pB.tile([P, P], F32, space="PSUM", tag="pd")
                    nc.tensor.matmul(ph[:], lhsT=w_s["w_dp"][:], rhs=xt_i,
                                     start=True, stop=True)
                    h0fm = cp.tile([P, P], F32, tag=f"h0fm{i}", name=f"h0fm{i}")
                    nc.scalar.activation(out=h0fm[:], in_=ph[:], func=AF.Gelu,
                                         bias=dpb_s[:])
                    h0fm_tiles.append(h0fm)
                    p2 = pB.tile([P, P], F32, space="PSUM", tag="pd")
                    nc.tensor.matmul(p2[:], lhsT=h0fm[:], rhs=w_s["w_g1l"][:],
                                     start=True, stop=True)
                    hl1bf = wp.tile([P, P], BF16, tag="hl1bf")
                    nc.vector.tensor_copy(out=hl1bf[:], in_=p2[:])
                    nc.sync.dma_start(out=hl1slab[i * P:(i + 1) * P, :], in_=hl1bf[:])

                # ---- AllGather 1 ----
                nc.gpsimd.collective_compute(
                    "AllGather", ALU.bypass,
                    replica_groups=[list(range(NCORE))],
                    ins=[hl1slab.opt()], outs=[hl1full.opt()])

                # ---- phase B2: shortcut + r-path, overlaps AllGather 1 ----
                for i in range(NT):
                    xt_i = xt_s[:, i * P:(i + 1) * P]
                    h0fm = h0fm_tiles[i]
                    p3 = pB.tile([P, P], F32, space="PSUM", tag="pd")
                    nc.tensor.matmul(p3[:], lhsT=h0fm[:], rhs=w_s["w_g1r"][:],
                                     start=True, stop=True)
                    h0r_s = wp.tile([P, P], F32, tag="h0rs")
                    nc.vector.tensor_tensor(out=h0r_s[:], in0=p3[:],
                                            in1=r_s["g1_lb"][:], op=ALU.add)
                    nc.sync.dma_start(out=h0r_d[i * P:(i + 1) * P, :], in_=h0r_s[:])
                    p4 = pB.tile([P, P], F32, space="PSUM", tag="pd")
                    nc.tensor.matmul(p4[:], lhsT=xt_i, rhs=w_s["w_sc"][:],
                                     start=True, stop=True)
                    sc_s = wp.tile([P, P], F32, tag="scs")
                    nc.vector.tensor_tensor(out=sc_s[:], in0=p4[:],
                                            in1=r_s["sc_b"][:], op=ALU.add)
                    nc.sync.dma_start(out=shct_d[i * P:(i + 1) * P, :], in_=sc_s[:])

                # ---- layer 1 aggregation + assembly ----
                h1fm_tiles = []
                for t in range(NT):
                    h1 = aggregate(hl1full, t)
                    h0r_t = wp.tile([P, D], F32, tag="h0rt")
                    nc.sync.dma_start(out=h0r_t[:], in_=h0r_d[t * P:(t + 1) * P, :])
                    nc.vector.tensor_tensor(out=h1[:], in0=h1[:], in1=h0r_t[:], op=ALU.add)
                    h1ln = layer_norm(h1, r_s["n1_g"], r_s["n1_b"])
                    h1g = wp.tile([P, D], F32, tag="hgel")
                    nc.scalar.activation(out=h1g[:], in_=h1ln[:], func=AF.Gelu)
                    tp = pB.tile([P, P], F32, space="PSUM", tag="pd")
                    nc.tensor.transpose(out=tp[:], in_=h1g[:], identity=ident[:])
                    h1fm = cp.tile([P, P], F32, tag=f"h1fm{t}", name=f"h1fm{t}")
                    nc.vector.tensor_copy(out=h1fm[:], in_=tp[:])
                    h1fm_tiles.append(h1fm)
                    p5 = pB.tile([P, P], F32, space="PSUM", tag="pd")
                    nc.tensor.matmul(p5[:], lhsT=h1fm[:], rhs=w_s["w_g2l"][:],
                                     start=True, stop=True)
                    hl2bf = wp.tile([P, P], BF16, tag="hl2bf")
                    nc.vector.tensor_copy(out=hl2bf[:], in_=p5[:])
                    nc.sync.dma_start(out=hl2slab[t * P:(t + 1) * P, :], in_=hl2bf[:])

                # ---- AllGather 2 ----
                nc.gpsimd.collective_compute(
                    "AllGather", ALU.bypass,
                    replica_groups=[list(range(NCORE))],
                    ins=[hl2slab.opt()], outs=[hl2full.opt()])

                # ---- layer 2 aggregation + assembly + output ----
                for t in range(NT):
                    h2 = aggregate(hl2full, t)
                    p6 = pB.tile([P, P], F32, space="PSUM", tag="pd")
                    nc.tensor.matmul(p6[:], lhsT=h1fm_tiles[t][:], rhs=w_s["w_g2r"][:],
                                     start=True, stop=True)
                    h1r_s = wp.tile([P, P], F32, tag="h1rs")
                    nc.vector.tensor_tensor(out=h1r_s[:], in0=p6[:],
                                            in1=r_s["g2_lb"][:], op=ALU.add)
                    nc.vector.tensor_tensor(out=h2[:], in0=h2[:], in1=h1r_s[:], op=ALU.add)
                    h2n = layer_norm(h2, r_s["n2_g"], r_s["n2_b"])
                    sh_t = wp.tile([P, D], F32, tag="sht")
                    nc.sync.dma_start(out=sh_t[:], in_=shct_d[t * P:(t + 1) * P, :])
                    nc.vector.tensor_tensor(out=h2n[:], in0=h2n[:], in1=sh_t[:], op=ALU.add)
                    o_t = wp.tile([P, D], F32, tag="ot")
                    nc.scalar.activation(out=o_t[:], in_=h2n[:], func=AF.Gelu)
                    nc.sync.dma_start(out=out[t * P:(t + 1) * P, :], in_=o_t[:])

    nc.compile()
    return nc


# ---------------------------------------------------------------------------
# host side: preprocessing + PJRT runner
# ---------------------------------------------------------------------------

class _Runner:
    """Reusable jitted PJRT executor for a compiled Bass module (axon)."""

    def __init__(self, nc, n_cores):
        import jax
        from jax.sharding import Mesh, PartitionSpec
        from jax.experimental.shard_map import shard_map
        from concourse.bass2jax import (_bass_exec_p, install_neuronx_cc_hook,
                                        partition_id_tensor)
        self.jax = jax
        install_neuronx_cc_hook()
        self.n_cores = n_cores
        pname = nc.partition_id_tensor.name if nc.partition_id_tensor else None
        in_names, out_names, out_avals, zero_outs = [], [], [], []
        for alloc in nc.m.functions[0].allocations:
            if not isinstance(alloc, mybir.MemoryLocationSet):
                continue
            name = alloc.memorylocations[0].name
            if alloc.kind == "ExternalInput":
                if name != pname:
                    in_names.append(name)
            elif alloc.kind == "ExternalOutput":
                shape = tuple(alloc.tensor_shape)
                dtype = mybir.dt.np(alloc.dtype)
                out_names.append(name)
                out_avals.append(jax.core.ShapedArray(shape, dtype))
                zero_outs.append(np.zeros(shape, dtype))
        self.in_names, self.out_names = in_names, out_names
        self.out_avals, self.zero_outs = out_avals, zero_outs
        n_params, n_outs = len(in_names), len(out_names)
        all_in = list(in_names) + list(out_names)
        if pname is not None:
            all_in.append(pname)

        def _body(*args):
            operands = list(args)
            if pname is not None:
                operands.append(partition_id_tensor())
            outs = _bass_exec_p.bind(
                *operands, out_avals=tuple(out_avals), in_names=tuple(all_in),
                out_names=tuple(out_names), lowering_input_output_aliases=(),
                sim_require_finite=False, sim_require_nnan=False, nc=nc)
            return tuple(outs)

        devices = jax.devices()[:n_cores]
        mesh = Mesh(np.asarray(devices), ("core",))
        self.mesh = mesh
        in_specs = (PartitionSpec("core"),) * (n_params + n_outs)
        out_specs = (PartitionSpec("core"),) * n_outs
        self.fn = jax.jit(
            shard_map(_body, mesh=mesh, in_specs=in_specs,
                      out_specs=out_specs, check_rep=False),
            keep_unused=True)

    def make_args(self, in_maps):
        """Concatenate per-core input maps into full arrays (host)."""
        n = self.n_cores
        args = [np.concatenate([np.asarray(in_maps[c][nm]) for c in range(n)], 0)
                for nm in self.in_names]
        args += [np.zeros((n * z.shape[0], *z.shape[1:]), z.dtype)
                 for z in self.zero_outs]
        return args

    def run_args(self, args):
        out_arrs = self.fn(*args)
        n = self.n_cores
        return [
            {nm: np.asarray(out_arrs[i]).reshape(n, *self.out_avals[i].shape)[c]
             for i, nm in enumerate(self.out_names)}
            for c in range(n)
        ]

    def run(self, in_maps):
        return self.run_args(self.make_args(in_maps))


_CACHE = {}


def _get_runner(cpt, reps=1):
    key = (cpt, reps)
    if key not in _CACHE:
        nc = _build_nc(cpt, reps)
        _CACHE[key] = _Runner(nc, NCORE)
    return _CACHE[key]


def _preprocess(x, edges, dp_w, dp_b, sc_w, sc_b, g1_lw, g1_lb, g1_rw, n1_g,
                n1_b, g2_lw, g2_lb, g2_rw, n2_g, n2_b):
    src = np.asarray(edges[0], dtype=np.int64)
    dst = np.asarray(edges[1], dtype=np.int64)
    x = np.asarray(x, dtype=np.float32)

    cnt = np.bincount(dst, minlength=N).astype(np.float32)
    inv = 1.0 / np.maximum(cnt, 1.0)
    # padded node id for the gathered tables; pair-packed row + parity
    pid = (src // OWN) * SLAB + (src % OWN)

    core_of = dst // OWN
    dloc_all = dst % OWN

    per_core = []
    cpt_needed = 0
    for c in range(NCORE):
        m = core_of == c
        idx = np.flatnonzero(m)
        o = idx[np.argsort(dloc_all[idx], kind="stable")]
        s_c, d_c = pid[o], dloc_all[o]
        tile_id = d_c // P
        counts = np.bincount(tile_id, minlength=NT)
        cpt_needed = max(cpt_needed, int(np.ceil(counts.max() / P)))
        per_core.append((s_c, d_c, counts))

    cpt = max(12, cpt_needed)

    ins = []
    for c in range(NCORE):
        s_c, d_c, counts = per_core[c]
        idx16 = np.zeros((NT, cpt * P), np.int16)
        dl2 = np.full((NT, 2 * cpt, P), -1.0, np.float32)
        starts = np.concatenate([[0], np.cumsum(counts)])
        for t in range(NT):
            lo, hi = starts[t], starts[t + 1]
            ne = hi - lo
            if ne == 0:
                continue
            p_t = s_c[lo:hi]
            r_t = (d_c[lo:hi] - t * P).astype(np.float32)
            idx16[t, :ne] = (p_t >> 1).astype(np.int16)
            par = (p_t & 1).astype(np.int64)
            j = np.arange(ne)
            dl2[t, 2 * (j // P) + par, j % P] = r_t

        # wrap idx16 into the Q7 layout: slot j -> partition j%16 (replicated
        # across the 8 groups of 16 partitions), column j//16; per-tile blocks
        w16 = idx16.reshape(NT, cpt * 8, 16).transpose(0, 2, 1)  # [t, u, s]
        blk = np.tile(w16, (1, 8, 1))                            # [t, 128, s]
        idx_arr = np.ascontiguousarray(
            blk.transpose(1, 0, 2).reshape(P, NT * cpt * 8))
        dl2_arr = np.ascontiguousarray(
            dl2.transpose(2, 0, 1).reshape(P, NT * 2 * cpt)
        ).astype(ml_dtypes.bfloat16)

        icnt = np.ones((P, NT), np.float32)
        base = c * OWN
        for t in range(NT):
            n0 = t * P
            n1 = min(n0 + P, OWN)
            icnt[:n1 - n0, t] = inv[base + n0:base + n1]

        x_slab = np.zeros((SLAB, D), np.float32)
        x_slab[:OWN] = x[base:base + OWN]

        ins.append({
            "x_t": np.ascontiguousarray(x_slab.T),
            "src_idx": idx_arr,
            "dl2_in": dl2_arr,
            "inv_cnt": icnt,
        })

    iota2 = np.tile(np.arange(P, dtype=np.float32), (P, 2 * cpt)).astype(
        ml_dtypes.bfloat16)
    shared = {
        "iota2_in": iota2,
        "w_dp": np.ascontiguousarray(np.asarray(dp_w, np.float32).T),
        "w_sc": np.ascontiguousarray(np.asarray(sc_w, np.float32).T),
        "w_g1l": np.ascontiguousarray(np.asarray(g1_lw, np.float32).T),
        "w_g1r": np.ascontiguousarray(np.asarray(g1_rw, np.float32).T),
        "w_g2l": np.ascontiguousarray(np.asarray(g2_lw, np.float32).T),
        "w_g2r": np.ascontiguousarray(np.asarray(g2_rw, np.float32).T),
        "dp_b": np.asarray(dp_b, np.float32).reshape(D, 1),
        "sc_b": np.tile(np.asarray(sc_b, np.float32), (P, 1)),
        "g1_lb": np.tile(np.asarray(g1_lb, np.float32), (P, 1)),
        "g2_lb": np.tile(np.asarray(g2_lb, np.float32), (P, 1)),
        "n1_g": np.tile(np.asarray(n1_g, np.float32), (P, 1)),
        "n1_b": np.tile(np.asarray(n1_b, np.float32), (P, 1)),
        "n2_g": np.tile(np.asarray(n2_g, np.float32), (P, 1)),
        "n2_b": np.tile(np.asarray(n2_b, np.float32), (P, 1)),
    }
    for m in ins:
        m.update(shared)
    return ins, cpt


def kernel(**inputs) -> np.ndarray:
    in_maps, cpt = _preprocess(**inputs)
    runner = _get_runner(cpt)
    res = runner.run(in_maps)
    return np.concatenate([res[c]["out"][:OWN] for c in range(NCORE)], axis=0)


# revision 12
# speedup vs baseline: 1.3502x; 1.3502x over previous
"""Trainium2 Bass kernel for nn_F2FBlock (2-layer SAGEConv GNN block).

Full inputs in, full output out. Internally: nodes sharded 6250/core across
8 NeuronCores (padded to 6272 = 49*128), edges sharded by dst ownership and
sorted by (dst tile, src parity, dst row) into 49 dst-tiles x CPT chunks of
128 edges (parity-split: chunks [0,CLO) hold even-src edges, [CLO,CPT) odd).

Aggregation: per dst tile ONE InstDMAGatherAnt (gpsimd dma_gather) fetches
CPT*128 pre-transformed rows from the pair-packed bf16 table
([NPAD/2, 256] = two nodes per 512B row; int16 row ids fit). One fused
is_equal builds all CPT one-hot scatter matrices; CPT PE matmuls segment-sum
the parity-correct 128-col halves into PSUM. LayerNorm uses DVE bn_stats/
bn_aggr + one batched ACT Rsqrt per layer (no per-tile activation-table
swaps). Dense path is bf16. The two conv layers exchange node features via
on-device AllGather.

reference math:
    shortcut = x @ sc_w.T + sc_b
    h = gelu(x @ dp_w.T + dp_b)
    h = mean_agg(h)@g1_lw.T + g1_lb + h@g1_rw.T          (SAGEConv 1)
    h = gelu(LN(h, n1_g, n1_b))
    h = mean_agg(h)@g2_lw.T + g2_lb + h@g2_rw.T          (SAGEConv 2)
    h = LN(h, n2_g, n2_b)
    out = gelu(h + shortcut)
where mean_agg(h)[i] = mean over {h[src] : (src,dst=i) in edges}.
Linearity lets us aggregate hl = h @ w_l.T and scale by 1/deg after.
"""

import numpy as np
import ml_dtypes

import concourse.bass as bass
import concourse.bacc as bacc
import concourse.tile as tile
import concourse.mybir as mybir
from concourse.masks import make_identity

P = 128
D = 128
N = 50000
NCORE = 8
OWN = N // NCORE            # 6250 owned nodes per core
NT = (OWN + P - 1) // P     # 49 dst tiles per core
SLAB = NT * P               # 6272 padded rows per core
NPAD = SLAB * NCORE         # 50176 rows in gathered tables
EPS = 1e-5

F32 = mybir.dt.float32
BF16 = mybir.dt.bfloat16
I16 = mybir.dt.int16
AF = mybir.ActivationFunctionType
ALU = mybir.AluOpType


def _build_nc(cpt: int, clo: int, reps: int = 1):
    """cpt = chunks per dst tile (clo even-parity + rest odd-parity)."""
    nidx = cpt * P
    nc = bacc.Bacc("TRN2", target_bir_lowering=False, debug=False,
                   num_devices=NCORE, num_swdge_queues=1)

    # ---- I/O ----
    x_t = nc.dram_tensor("x_t", [P, SLAB], BF16, kind="ExternalInput")
    src_idx = nc.dram_tensor("src_idx", [P, NT * cpt * 8], I16,
                             kind="ExternalInput")
    dl_in = nc.dram_tensor("dl_in", [P, NT * cpt], BF16, kind="ExternalInput")
    iota_in = nc.dram_tensor("iota_in", [P, cpt * P], BF16,
                             kind="ExternalInput")
    inv_cnt = nc.dram_tensor("inv_cnt", [P, NT], F32, kind="ExternalInput")
    # weight matrices, already transposed to [fin, fout] on host (bf16)
    w_names = ["w_dp", "w_sc", "w_g1l", "w_g1r", "w_g2l", "w_g2r"]
    w_in = {n: nc.dram_tensor(n, [D, D], BF16, kind="ExternalInput")
            for n in w_names}
    dp_b = nc.dram_tensor("dp_b", [D, 1], F32, kind="ExternalInput")
    eps_in = nc.dram_tensor("eps_in", [P, 1], F32, kind="ExternalInput")
    # feature-axis vectors replicated to [P, D] on host
    r_names = ["sc_b", "g1_lb", "g2_lb", "n1_g", "n1_b", "n2_g", "n2_b"]
    r_in = {n: nc.dram_tensor(n, [P, D], F32, kind="ExternalInput")
            for n in r_names}
    out = nc.dram_tensor("out", [SLAB, D], F32, kind="ExternalOutput")

    with tile.TileContext(nc) as tc:
        with (
            tc.tile_pool(name="const", bufs=1) as cp,
            tc.tile_pool(name="work", bufs=4) as wp,
            tc.tile_pool(name="msgs", bufs=4) as mp,
            tc.tile_pool(name="oneh", bufs=4) as op_,
            tc.tile_pool(name="small", bufs=4) as sp,
            tc.tile_pool(name="psA", bufs=2, space="PSUM") as pA,
            tc.tile_pool(name="psB", bufs=4, space="PSUM") as pB,
            tc.tile_pool(name="dram", bufs=1, space="DRAM") as dp_,
        ):
            # ---- constants into SBUF ----
            xt_s = cp.tile([P, SLAB], BF16, tag="xt")
            nc.sync.dma_start(out=xt_s[:], in_=x_t[:])
            si_s = cp.tile([P, NT * cpt * 8], I16, tag="si")
            nc.sync.dma_start(out=si_s[:], in_=src_idx[:])
            dl_s = cp.tile([P, NT * cpt], BF16, tag="dl")
            nc.sync.dma_start(out=dl_s[:], in_=dl_in[:])
            io_s = cp.tile([P, cpt * P], BF16, tag="io")
            nc.sync.dma_start(out=io_s[:], in_=iota_in[:])
            ic_s = cp.tile([P, NT], F32, tag="ic")
            nc.sync.dma_start(out=ic_s[:], in_=inv_cnt[:])
            w_s = {}
            for n in w_names:
                w_s[n] = cp.tile([D, D], BF16, tag=n, name=n)
                nc.sync.dma_start(out=w_s[n][:], in_=w_in[n][:])
            dpb_s = cp.tile([D, 1], F32, tag="dpb")
            nc.sync.dma_start(out=dpb_s[:], in_=dp_b[:])
            eps_s = cp.tile([P, 1], F32, tag="eps")
            nc.sync.dma_start(out=eps_s[:], in_=eps_in[:])
            r_s = {}
            for n in r_names:
                r_s[n] = cp.tile([P, D], F32, tag=n, name=n)
                nc.sync.dma_start(out=r_s[n][:], in_=r_in[n][:])
            ident = cp.tile([P, P], F32, tag="ident")
            make_identity(nc, ident[:])

            # ---- persistent SBUF feature buffers (subtile-dep tracked) ----
            h0fm_b = cp.tile([P, SLAB], BF16, tag="h0fm")   # gelu(dp) f-major
            h1fm_b = cp.tile([P, SLAB], BF16, tag="h1fm")   # post-LN1 f-major
            h0r_b = cp.tile([P, SLAB], BF16, tag="h0r")     # h0@g1r + g1_lb
            h1r_b = cp.tile([P, SLAB], BF16, tag="h1r")     # h1@g2r + g2_lb
            shct_b = cp.tile([P, SLAB], BF16, tag="shct")   # x@sc + sc_b
            hbuf = cp.tile([P, SLAB], F32, tag="hbuf")      # node-major accum
            st6 = cp.tile([P, NT * 6], F32, tag="st6")
            mvb = cp.tile([P, NT * 2], F32, tag="mvb")      # (mean,var) pairs
            rstd_b = cp.tile([P, NT * 2], F32, tag="rstd")  # rsqrt(mvb+eps)
            nmr_b = cp.tile([P, NT], F32, tag="nmr")        # -mean*rstd

            # internal DRAM (tables pair-packed: [NPAD//2, 256])
            hl1slab = dp_.tile([SLAB, D], BF16)
            hl1full = dp_.tile([NPAD // 2, 2 * D], BF16)
            hl2slab = dp_.tile([SLAB, D], BF16)
            hl2full = dp_.tile([NPAD // 2, 2 * D], BF16)

            def aggregate(table, t, other_b):
                """hbuf[:, tP:(t+1)P] = mean_agg(table)[tile t] + other_b
                slice; then bn stats into st6/mvb."""
                msgs = mp.tile([P, cpt * 2 * D], BF16, tag="msgs")
                nc.gpsimd.dma_gather(
                    out_ap=msgs[:].rearrange("p (c e) -> p c e", e=2 * D),
                    in_ap=table[:],
                    idxs_ap=si_s[:, t * cpt * 8:(t + 1) * cpt * 8],
                    num_idxs=nidx,
                    num_idxs_reg=nidx,
                    elem_size=2 * D,
                    queue_num=0,
                    single_packet=False,
                )
                w_all = op_.tile([P, cpt * P], BF16, tag="oneh")
                nc.vector.tensor_tensor(
                    out=w_all[:].rearrange("p (c e) -> p c e", e=P),
                    in0=dl_s[:, t * cpt:(t + 1) * cpt]
                        .to_broadcast([P, cpt, P]),
                    in1=io_s[:].rearrange("p (c e) -> p c e", e=P),
                    op=ALU.is_equal)
                ps = pA.tile([P, D], F32, space="PSUM", tag="agg")
                for k in range(cpt):
                    off = k * 2 * D + (D if k >= clo else 0)
                    nc.tensor.matmul(ps[:], lhsT=w_all[:, k * P:(k + 1) * P],
                                     rhs=msgs[:, off:off + D],
                                     start=(k == 0), stop=(k == cpt - 1))
                hsl = hbuf[:, t * P:(t + 1) * P]
                nc.vector.scalar_tensor_tensor(
                    out=hsl, in0=ps[:], scalar=ic_s[:, t:t + 1],
                    in1=other_b[:, t * P:(t + 1) * P],
                    op0=ALU.mult, op1=ALU.add)
                nc.vector.bn_stats(out=st6[:, t * 6:(t + 1) * 6], in_=hsl)
                nc.vector.bn_aggr(out=mvb[:, t * 2:(t + 1) * 2],
                                  in_=st6[:, t * 6:(t + 1) * 6])

            def ln_batch(gamma, beta):
                """Normalize hbuf in place using mvb stats (all NT tiles)."""
                nc.scalar.activation(out=rstd_b[:], in_=mvb[:], func=AF.Sqrt,
                                     bias=eps_s[:])
                nc.vector.reciprocal(out=rstd_b[:], in_=rstd_b[:])
                for t in range(NT):
                    nc.vector.scalar_tensor_tensor(
                        out=nmr_b[:, t:t + 1], in0=mvb[:, 2 * t:2 * t + 1],
                        scalar=-1.0, in1=rstd_b[:, 2 * t + 1:2 * t + 2],
                        op0=ALU.mult, op1=ALU.mult)
                for t in range(NT):
                    hsl = hbuf[:, t * P:(t + 1) * P]
                    nc.scalar.activation(
                        out=hsl, in_=hsl, func=AF.Identity,
                        scale=rstd_b[:, 2 * t + 1:2 * t + 2],
                        bias=nmr_b[:, t:t + 1])
                g3 = r_s[gamma][:].rearrange("p (o d) -> p o d", o=1) \
                    .broadcast_to([P, NT, D])
                b3 = r_s[beta][:].rearrange("p (o d) -> p o d", o=1) \
                    .broadcast_to([P, NT, D])
                h3 = hbuf[:].rearrange("p (t d) -> p t d", d=D)
                nc.vector.tensor_tensor(out=h3, in0=h3, in1=g3, op=ALU.mult)
                nc.vector.tensor_tensor(out=h3, in0=h3, in1=b3, op=ALU.add)

            for _rep in range(reps):
                # ---- phase B1: hl1 chain only (feeds AllGather 1 asap) ----
                for i in range(NT):
                    xt_i = xt_s[:, i * P:(i + 1) * P]
                    ph = pB.tile([P, P], F32, space="PSUM", tag="pd")
                    nc.tensor.matmul(ph[:], lhsT=w_s["w_dp"][:], rhs=xt_i,
                                     start=True, stop=True)
                    h0sl = h0fm_b[:, i * P:(i + 1) * P]
                    nc.scalar.activation(out=h0sl, in_=ph[:], func=AF.Gelu,
                                         bias=dpb_s[:])
                    p2 = pB.tile([P, P], F32, space="PSUM", tag="pd")
                    nc.tensor.matmul(p2[:], lhsT=h0sl, rhs=w_s["w_g1l"][:],
                                     start=True, stop=True)
                    hl1bf = wp.tile([P, P], BF16, tag="hl1bf")
                    nc.vector.tensor_copy(out=hl1bf[:], in_=p2[:])
                    nc.sync.dma_start(out=hl1slab[i * P:(i + 1) * P, :],
                                      in_=hl1bf[:])

                # ---- AllGather 1 ----
                nc.gpsimd.collective_compute(
                    "AllGather", ALU.bypass,
                    replica_groups=[list(range(NCORE))],
                    ins=[hl1slab.opt()], outs=[hl1full.opt()])

                # ---- phase B2: h0r + shortcut, overlaps AllGather 1 ----
                for i in range(NT):
                    xt_i = xt_s[:, i * P:(i + 1) * P]
                    p3 = pB.tile([P, P], F32, space="PSUM", tag="pd")
                    nc.tensor.matmul(p3[:], lhsT=h0fm_b[:, i * P:(i + 1) * P],
                                     rhs=w_s["w_g1r"][:],
                                     start=True, stop=True)
                    nc.vector.tensor_tensor(out=h0r_b[:, i * P:(i + 1) * P],
                                            in0=p3[:], in1=r_s["g1_lb"][:],
                                            op=ALU.add)
                    p4 = pB.tile([P, P], F32, space="PSUM", tag="pd")
                    nc.tensor.matmul(p4[:], lhsT=xt_i, rhs=w_s["w_sc"][:],
                                     start=True, stop=True)
                    nc.vector.tensor_tensor(out=shct_b[:, i * P:(i + 1) * P],
                                            in0=p4[:], in1=r_s["sc_b"][:],
                                            op=ALU.add)

                # ---- layer 1 aggregation ----
                for t in range(NT):
                    aggregate(hl1full, t, h0r_b)
                # ---- LN1 + gelu (batched) ----
                ln_batch("n1_g", "n1_b")
                nc.scalar.activation(out=hbuf[:], in_=hbuf[:], func=AF.Gelu)
                # ---- transpose + hl2 chain (feeds AllGather 2) ----
                for t in range(NT):
                    tp = pB.tile([P, P], F32, space="PSUM", tag="pd")
                    nc.tensor.transpose(out=tp[:],
                                        in_=hbuf[:, t * P:(t + 1) * P],
                                        identity=ident[:])
                    h1sl = h1fm_b[:, t * P:(t + 1) * P]
                    nc.vector.tensor_copy(out=h1sl, in_=tp[:])
                    p5 = pB.tile([P, P], F32, space="PSUM", tag="pd")
                    nc.tensor.matmul(p5[:], lhsT=h1sl, rhs=w_s["w_g2l"][:],
                                     start=True, stop=True)
                    hl2bf = wp.tile([P, P], BF16, tag="hl2bf")
                    nc.vector.tensor_copy(out=hl2bf[:], in_=p5[:])
                    nc.sync.dma_start(out=hl2slab[t * P:(t + 1) * P, :],
                                      in_=hl2bf[:])

                # ---- AllGather 2 ----
                nc.gpsimd.collective_compute(
                    "AllGather", ALU.bypass,
                    replica_groups=[list(range(NCORE))],
                    ins=[hl2slab.opt()], outs=[hl2full.opt()])

                # ---- h1r path (overlaps AllGather 2) ----
                for t in range(NT):
                    p6 = pB.tile([P, P], F32, space="PSUM", tag="pd")
                    nc.tensor.matmul(p6[:], lhsT=h1fm_b[:, t * P:(t + 1) * P],
                                     rhs=w_s["w_g2r"][:],
                                     start=True, stop=True)
                    nc.vector.tensor_tensor(out=h1r_b[:, t * P:(t + 1) * P],
                                            in0=p6[:], in1=r_s["g2_lb"][:],
                                            op=ALU.add)

                # ---- layer 2 aggregation ----
                for t in range(NT):
                    aggregate(hl2full, t, h1r_b)
                # ---- LN2 + shortcut + gelu (batched) + output ----
                ln_batch("n2_g", "n2_b")
                h3 = hbuf[:].rearrange("p (t d) -> p t d", d=D)
                s3 = shct_b[:].rearrange("p (t d) -> p t d", d=D)
                nc.vector.tensor_tensor(out=h3, in0=h3, in1=s3, op=ALU.add)
                nc.scalar.activation(out=hbuf[:], in_=hbuf[:], func=AF.Gelu)
                for t in range(NT):
                    nc.sync.dma_start(out=out[t * P:(t + 1) * P, :],
                                      in_=hbuf[:, t * P:(t + 1) * P])

    nc.compile()
    return nc


# ---------------------------------------------------------------------------
# host side: preprocessing + PJRT runner
# ---------------------------------------------------------------------------

class _Runner:
    """Reusable jitted PJRT executor for a compiled Bass module (axon)."""

    def __init__(self, nc, n_cores):
        import jax
        from jax.sharding import Mesh, PartitionSpec
        from jax.experimental.shard_map import shard_map
        from concourse.bass2jax import (_bass_exec_p, install_neuronx_cc_hook,
                                        partition_id_tensor)
        self.jax = jax
        install_neuronx_cc_hook()
        self.n_cores = n_cores
        pname = nc.partition_id_tensor.name if nc.partition_id_tensor else None
        in_names, out_names, out_avals, zero_outs = [], [], [], []
        for alloc in nc.m.functions[0].allocations:
            if not isinstance(alloc, mybir.MemoryLocationSet):
                continue
            name = alloc.memorylocations[0].name
            if alloc.kind == "ExternalInput":
                if name != pname:
                    in_names.append(name)
            elif alloc.kind == "ExternalOutput":
                shape = tuple(alloc.tensor_shape)
                dtype = mybir.dt.np(alloc.dtype)
                out_names.append(name)
                out_avals.append(jax.core.ShapedArray(shape, dtype))
                zero_outs.append(np.zeros(shape, dtype))
        self.in_names, self.out_names = in_names, out_names
        self.out_avals, self.zero_outs = out_avals, zero_outs
        n_params, n_outs = len(in_names), len(out_names)
        all_in = list(in_names) + list(out_names)
        if pname is not None:
            all_in.append(pname)

        def _body(*args):
            operands = list(args)
            if pname is not None:
                operands.append(partition_id_tensor())
            outs = _bass_exec_p.bind(
                *operands, out_avals=tuple(out_avals), in_names=tuple(all_in),
                out_names=tuple(out_names), lowering_input_output_aliases=(),
                sim_require_finite=False, sim_require_nnan=False, nc=nc)
            return tuple(outs)

        devices = jax.devices()[:n_cores]
        mesh = Mesh(np.asarray(devices), ("core",))
        self.mesh = mesh
        in_specs = (PartitionSpec("core"),) * (n_params + n_outs)
        out_specs = (PartitionSpec("core"),) * n_outs
        self.fn = jax.jit(
            shard_map(_body, mesh=mesh, in_specs=in_specs,
                      out_specs=out_specs, check_rep=False),
            keep_unused=True)

    def make_args(self, in_maps):
        n = self.n_cores
        args = [np.concatenate([np.asarray(in_maps[c][nm]) for c in range(n)], 0)
                for nm in self.in_names]
        args += [np.zeros((n * z.shape[0], *z.shape[1:]), z.dtype)
                 for z in self.zero_outs]
        return args

    def run_args(self, args):
        out_arrs = self.fn(*args)
        n = self.n_cores
        return [
            {nm: np.asarray(out_arrs[i]).reshape(n, *self.out_avals[i].shape)[c]
             for i, nm in enumerate(self.out_names)}
            for c in range(n)
        ]

    def run(self, in_maps):
        return self.run_args(self.make_args(in_maps))


_CACHE = {}


def _get_runner(cpt, clo, reps=1):
    key = (cpt, clo, reps)
    if key not in _CACHE:
        nc = _build_nc(cpt, clo, reps)
        _CACHE[key] = _Runner(nc, NCORE)
    return _CACHE[key]


def _preprocess(x, edges, dp_w, dp_b, sc_w, sc_b, g1_lw, g1_lb, g1_rw, n1_g,
                n1_b, g2_lw, g2_lb, g2_rw, n2_g, n2_b):
    src = np.asarray(edges[0], dtype=np.int64)
    dst = np.asarray(edges[1], dtype=np.int64)
    x = np.asarray(x, dtype=np.float32)

    cnt = np.bincount(dst, minlength=N).astype(np.float32)
    inv = 1.0 / np.maximum(cnt, 1.0)
    # padded node id; pair-packed row = pid>>1, parity = pid&1
    pid = (src // OWN) * SLAB + (src % OWN)

    core_of = dst // OWN
    dloc_all = dst % OWN

    per_core = []
    clo_need = chi_need = 0
    for c in range(NCORE):
        m = core_of == c
        idx = np.flatnonzero(m)
        s_c, d_c = pid[idx], dloc_all[idx]
        tile_id = d_c // P
        pr = (s_c & 1)
        nlo = np.bincount(tile_id[pr == 0], minlength=NT)
        nhi = np.bincount(tile_id[pr == 1], minlength=NT)
        clo_need = max(clo_need, int(np.ceil(nlo.max() / P)))
        chi_need = max(chi_need, int(np.ceil(nhi.max() / P)))
        per_core.append((s_c, d_c, tile_id, pr))

    clo, chi = max(1, clo_need), max(1, chi_need)
    cpt = clo + chi

    ins = []
    for c in range(NCORE):
        s_c, d_c, tile_id, pr = per_core[c]
        idx16 = np.zeros((NT, cpt * P), np.int16)
        dl = np.full((NT, cpt * P), -1.0, np.float32)
        for t in range(NT):
            for h, base in ((0, 0), (1, clo * P)):
                sel = (tile_id == t) & (pr == h)
                ne = int(sel.sum())
                if ne == 0:
                    continue
                idx16[t, base:base + ne] = (s_c[sel] >> 1).astype(np.int16)
                dl[t, base:base + ne] = (d_c[sel] - t * P).astype(np.float32)

        # wrap idx16 into the Q7 layout: slot j -> partition j%16 (replicated
        # across the 8 groups of 16 partitions), column j//16; per-tile blocks
        w16 = idx16.reshape(NT, cpt * 8, 16).transpose(0, 2, 1)  # [t, u, s]
        blk = np.tile(w16, (1, 8, 1))                            # [t, 128, s]
        idx_arr = np.ascontiguousarray(
            blk.transpose(1, 0, 2).reshape(P, NT * cpt * 8))
        # dl per chunk: [128, NT*cpt], col t*cpt+k, partition = slot % 128
        dl_arr = np.ascontiguousarray(
            dl.reshape(NT, cpt, P).transpose(2, 0, 1).reshape(P, NT * cpt)
        ).astype(ml_dtypes.bfloat16)

        icnt = np.ones((P, NT), np.float32)
        base_n = c * OWN
        for t in range(NT):
            n0 = t * P
            n1 = min(n0 + P, OWN)
            icnt[:n1 - n0, t] = inv[base_n + n0:base_n + n1]

        x_slab = np.zeros((SLAB, D), np.float32)
        x_slab[:OWN] = x[base_n:base_n + OWN]

        ins.append({
            "x_t": np.ascontiguousarray(x_slab.T).astype(ml_dtypes.bfloat16),
            "src_idx": idx_arr,
            "dl_in": dl_arr,
            "inv_cnt": icnt,
        })

    iota = np.tile(np.arange(P, dtype=np.float32),
                   (P, cpt)).astype(ml_dtypes.bfloat16)
    bf = lambda a: np.ascontiguousarray(np.asarray(a, np.float32).T).astype(
        ml_dtypes.bfloat16)
    shared = {
        "iota_in": iota,
        "w_dp": bf(dp_w),
        "w_sc": bf(sc_w),
        "w_g1l": bf(g1_lw),
        "w_g1r": bf(g1_rw),
        "w_g2l": bf(g2_lw),
        "w_g2r": bf(g2_rw),
        "dp_b": np.asarray(dp_b, np.float32).reshape(D, 1),
        "eps_in": np.full((P, 1), EPS, np.float32),
        "sc_b": np.tile(np.asarray(sc_b, np.float32), (P, 1)),
        "g1_lb": np.tile(np.asarray(g1_lb, np.float32), (P, 1)),
        "g2_lb": np.tile(np.asarray(g2_lb, np.float32), (P, 1)),
        "n1_g": np.tile(np.asarray(n1_g, np.float32), (P, 1)),
        "n1_b": np.tile(np.asarray(n1_b, np.float32), (P, 1)),
        "n2_g": np.tile(np.asarray(n2_g, np.float32), (P, 1)),
        "n2_b": np.tile(np.asarray(n2_b, np.float32), (P, 1)),
    }
    for m in ins:
        m.update(shared)
    return ins, cpt, clo


def kernel(**inputs) -> np.ndarray:
    in_maps, cpt, clo = _preprocess(**inputs)
    runner = _get_runner(cpt, clo)
    res = runner.run(in_maps)
    return np.concatenate([res[c]["out"][:OWN] for c in range(NCORE)], axis=0)
